# revision 1
# baseline (speedup 1.0000x reference)
"""MoE gate kernel for Trainium2 (8 NeuronCores, SPMD).

Computes, for hidden_states [4, 4096, 2048] and gate weight [64, 2048]:
  logits = x @ W^T          (T=16384 tokens, E=64 experts)
  scores = softmax(logits)
  topk_weight, topk_idx = top_k(scores, 8), weights renormalized over the top-8
  row_idx = arange(T*K).reshape(K, T).T   (data independent)

Sharding: tokens split evenly across 8 cores (2048 tokens/core); the gate
weight is replicated.  The host pre-transposes each token shard to [H, Tc]
so the device streams it contiguously with h on the SBUF partition axis
(the tensor-engine contraction axis) -- no on-device transpose of the big
tensor is needed.

Matmul precision: fp16 hi/lo 3-pass.  The host splits x (pre-scaled by 2^8)
and W (pre-scaled by 2^10) each into a coarse fp16 value plus an fp16
residual scaled by 2^11.  Three full-rate fp16 matmul passes
(XH*WH at scale 1; XH*WL + XL*WH at scale 2^-11, dropping the ~2^-22 XL*WL
term) give ~fp32-accuracy logits (max abs err ~3e-6, zero top-8 order flips
vs the fp32 jax reference on the fixed problem input) while streaming the
same total bytes as fp32 (2 x 2B per element) and running the PE at
1 cycle/row instead of fp32's 4.  The pre-scales are powers of two
(lossless) chosen so no fp16 value lands in the subnormal range; the
combined 2^-18 descale is folded into the softmax exp's scale argument.

Per core: logits^T [64, 512] accumulated in two PSUM banks over 16 h-chunks,
recombined with one DVE op, PE-transposed to [tokens, experts], hardware
top-8 via DVE max / max_index, softmax weights from exp over just the top-8
(the full-softmax normalizer cancels in the reference's renormalization).

Pipelining: inputs stream as 0.5MB quarter-tiles split across BOTH HWDGE
rings -- xh on SP, xl on ACT (only SP/ACT can issue DMAs) -- so the two
input streams run concurrently and xh (needed first, for pass A) arrives
at ~2x rate.  Outputs and the small weight loads also ride the ACT ring
(tiny next to the 1MB/block xl stream).  Best measured ~47us/core per
full pass on HW (min-based slope), i.e. at the ~47us pure-HBM-stream
floor; single-ring input streaming measured ~69us.
"""

import numpy as np

# -- problem constants (hardcoded per contract) --
B, S, H = 4, 4096, 2048
T = B * S                  # 16384 tokens
E = 64                     # experts
K = 8                      # top-k
N_CORES = 8
TC = T // N_CORES          # 2048 tokens per core
TB = 512                   # tokens per block (one PSUM bank of logits^T)
NB = TC // TB              # 4 blocks
P = 128                    # SBUF partitions
CH = H // P                # 16 h-chunks
NT = TB // P               # 4 token sub-tiles per block

SX = 2.0 ** 8              # x pre-scale (keeps fp16 out of subnormals)
SW = 2.0 ** 10             # w pre-scale
SL = 2.0 ** 11             # lo-part scale
DESCALE = 1.0 / (SX * SW)  # folded into the exp's scale argument

_CACHE = {}


def _build_program(repeats=1, loop_iters=1):
    import concourse.bacc as bacc
    import concourse.mybir as mybir
    import concourse.tile as tile
    from concourse.mybir import dt, ActivationFunctionType as AFT, AluOpType
    from contextlib import ExitStack, nullcontext

    f32 = dt.float32
    f16 = dt.float16
    u32 = dt.uint32

    nc = bacc.Bacc("TRN2", target_bir_lowering=False, debug=False,
                   num_devices=N_CORES)

    xh = nc.dram_tensor("xh", [H, TC], f16, kind="ExternalInput")
    xl = nc.dram_tensor("xl", [H, TC], f16, kind="ExternalInput")
    wh = nc.dram_tensor("wh", [H, E], f16, kind="ExternalInput")
    wl = nc.dram_tensor("wl", [H, E], f16, kind="ExternalInput")
    ident = nc.dram_tensor("ident", [E, E], f32, kind="ExternalInput")
    out_w = nc.dram_tensor("out_w", [TC, K], f32, kind="ExternalOutput")
    out_i = nc.dram_tensor("out_i", [TC, K], u32, kind="ExternalOutput")

    with tile.TileContext(nc) as tc:
        with ExitStack() as ctx:
            wpool = ctx.enter_context(tc.tile_pool(name="w", bufs=1))
            xpool = ctx.enter_context(tc.tile_pool(name="x", bufs=4))
            lgpool = ctx.enter_context(tc.tile_pool(name="lg", bufs=2,
                                                    space="PSUM"))
            tpool = ctx.enter_context(tc.tile_pool(name="tp", bufs=2,
                                                   space="PSUM"))
            scpool = ctx.enter_context(tc.tile_pool(name="sc", bufs=2))
            stpool = ctx.enter_context(tc.tile_pool(name="st", bufs=2))
            smpool = ctx.enter_context(tc.tile_pool(name="sm", bufs=4))

            # weights + identity ride the ACT HWDGE ring so they load in
            # parallel with the first x quarter on the SP ring
            wh_t = wpool.tile([P, CH, E], f16)
            nc.scalar.dma_start(wh_t[:], wh.rearrange("(c p) e -> p c e", p=P))
            wl_t = wpool.tile([P, CH, E], f16)
            nc.scalar.dma_start(wl_t[:], wl.rearrange("(c p) e -> p c e", p=P))
            id_tile = wpool.tile([E, E], f32)
            nc.scalar.dma_start(id_tile[:], ident[:])

            QC = CH // 4           # chunks per input quarter-tile
            loop_cm = (tc.For_i(0, loop_iters, 1) if loop_iters > 1
                       else nullcontext())
            with loop_cm:
                for rep in range(repeats):
                    # all input triggers first: DMA triggers retire in program
                    # order on their issuing engine, so emitting them before
                    # any compute keeps both rings streaming continuously --
                    # xl triggers must never queue behind ACT's copies/exp
                    # (those depend on the whole compute chain of a block)
                    xh_qb, xl_qb = [], []
                    for b in range(NB):
                        tsl = slice(b * TB, (b + 1) * TB)
                        xh_src = xh[:, tsl].rearrange("(c p) t -> p c t", p=P)
                        xl_src = xl[:, tsl].rearrange("(c p) t -> p c t", p=P)
                        xh_q = []
                        xl_q = []
                        for q in range(4):
                            csl = slice(q * QC, (q + 1) * QC)
                            th = xpool.tile([P, QC, TB], f16, tag=f"xh{q}")
                            nc.sync.dma_start(th[:], xh_src[:, csl, :])
                            xh_q.append(th)
                            # xl rides the ACT HWDGE ring (only SP/ACT can
                            # issue DMAs): two concurrent input streams
                            tl_ = xpool.tile([P, QC, TB], f16, tag=f"xl{q}")
                            nc.scalar.dma_start(tl_[:], xl_src[:, csl, :])
                            xl_q.append(tl_)
                        xh_qb.append(xh_q)
                        xl_qb.append(xl_q)

                for b in range(NB):
                    xh_q, xl_q = xh_qb[b], xl_qb[b]

                    # pass A: XH*WH (scale 1); pass B: XH*WL + XL*WH (scale 2^-11)
                    ps_a = lgpool.tile([E, TB], f32, tag="psA")
                    for c in range(CH):
                        nc.tensor.matmul(ps_a[:], wh_t[:, c, :],
                                         xh_q[c // QC][:, c % QC, :],
                                         start=(c == 0), stop=(c == CH - 1))
                    ps_b = lgpool.tile([E, TB], f32, tag="psB")
                    for c in range(CH):
                        nc.tensor.matmul(ps_b[:], wl_t[:, c, :],
                                         xh_q[c // QC][:, c % QC, :],
                                         start=(c == 0), stop=False)
                        nc.tensor.matmul(ps_b[:], wh_t[:, c, :],
                                         xl_q[c // QC][:, c % QC, :],
                                         start=False, stop=(c == CH - 1))

                    sc_a = scpool.tile([E, TB], f32, tag="scA")
                    nc.scalar.copy(sc_a[:], ps_a[:])
                    # scores^T = A + B * 2^-11   (still scaled by 2^18 overall)
                    scT = scpool.tile([E, TB], f32, tag="scT")
                    nc.vector.scalar_tensor_tensor(
                        scT[:], ps_b[:], float(1.0 / SL), sc_a[:],
                        op0=AluOpType.mult, op1=AluOpType.add)

                    # transpose to [tokens, experts] in PSUM, then to SBUF
                    ps_sc = tpool.tile([P, NT * E], f32, tag="pssc")
                    for k in range(NT):
                        nc.tensor.transpose(ps_sc[:, k * E:(k + 1) * E],
                                            scT[:, k * P:(k + 1) * P],
                                            id_tile[:])
                    sc = scpool.tile([P, NT * E], f32, tag="sc")
                    nc.scalar.copy(sc[:], ps_sc[:])

                    w_st = stpool.tile([P, NT, K], f32, tag="wst")
                    i_st = stpool.tile([P, NT, K], u32, tag="ist")
                    for k in range(NT):
                        sck = sc[:, k * E:(k + 1) * E]
                        mx = smpool.tile([P, K], f32, tag="mx")
                        nc.vector.max(mx[:], sck)
                        nc.vector.max_index(i_st[:, k, :], mx[:], sck)
                        ex = smpool.tile([P, K], f32, tag="ex")
                        den = smpool.tile([P, 1], f32, tag="den")
                        # exp(score * 2^-18): undo the hi/lo pre-scales here
                        nc.scalar.activation(ex[:], mx[:], AFT.Exp,
                                             scale=float(DESCALE),
                                             accum_out=den[:])
                        rd = smpool.tile([P, 1], f32, tag="rd")
                        nc.vector.reciprocal(rd[:], den[:])
                        nc.vector.tensor_scalar_mul(w_st[:, k, :], ex[:],
                                                    rd[:, 0:1])

                    dst_w = out_w[b * TB:(b + 1) * TB, :].rearrange(
                        "(n p) k -> p n k", p=P)
                    dst_i = out_i[b * TB:(b + 1) * TB, :].rearrange(
                        "(n p) k -> p n k", p=P)
                    # outputs go out on the ACT HWDGE ring: the SP ring must stay
                    # free for the next block's input loads (FIFO per ring)
                    nc.scalar.dma_start(dst_w, w_st[:])
                    nc.scalar.dma_start(dst_i, i_st[:])

    nc.compile()
    return nc


def _get_program_loop(loop_iters):
    key = ("loop", loop_iters)
    if key not in _CACHE:
        _CACHE[key] = _build_program(loop_iters=loop_iters)
    return _CACHE[key]


def _get_program(repeats=1):
    key = ("nc", repeats)
    if key not in _CACHE:
        _CACHE[key] = _build_program(repeats)
    return _CACHE[key]


def _prepare_inputs(hidden_states, weight):
    x = np.asarray(hidden_states, dtype=np.float32).reshape(T, H)
    w = np.asarray(weight, dtype=np.float32)

    xs = x * np.float32(SX)
    xh = xs.astype(np.float16)
    xl = ((xs - xh.astype(np.float32)) * np.float32(SL)).astype(np.float16)
    ws = w * np.float32(SW)
    wh = ws.astype(np.float16)
    wl = ((ws - wh.astype(np.float32)) * np.float32(SL)).astype(np.float16)

    xhT = np.ascontiguousarray(xh.T)             # [H, T] fp16
    xlT = np.ascontiguousarray(xl.T)
    whT = np.ascontiguousarray(wh.T)             # [H, E] fp16
    wlT = np.ascontiguousarray(wl.T)
    ident = np.eye(E, dtype=np.float32)

    return [
        {"xh": np.ascontiguousarray(xhT[:, i * TC:(i + 1) * TC]),
         "xl": np.ascontiguousarray(xlT[:, i * TC:(i + 1) * TC]),
         "wh": whT, "wl": wlT, "ident": ident}
        for i in range(N_CORES)
    ]


def _enable_jax_compile_cache():
    # Persistent executable cache: lets repeat invocations (fresh processes)
    # skip the multi-minute neuronx compile when the backend supports
    # executable serialization.  Harmless no-op otherwise.
    try:
        import os
        import jax
        jax.config.update("jax_compilation_cache_dir",
                          os.path.expanduser("~/.cache/jax_bass_cache"))
        jax.config.update("jax_persistent_cache_min_entry_size_bytes", -1)
        jax.config.update("jax_persistent_cache_min_compile_time_secs", 0)
    except Exception:
        pass


def kernel(hidden_states, weight):
    from concourse.bass_utils import run_bass_kernel_spmd

    _enable_jax_compile_cache()
    in_maps = _prepare_inputs(hidden_states, weight)
    nc = _get_program()
    res = run_bass_kernel_spmd(nc, in_maps, list(range(N_CORES))).results

    topk_w = np.concatenate([res[i]["out_w"] for i in range(N_CORES)], axis=0)
    topk_i = np.concatenate([res[i]["out_i"] for i in range(N_CORES)],
                            axis=0).astype(np.int32)
    row_idx = np.arange(T * K, dtype=np.int32).reshape(K, T).T
    return topk_i, topk_w.astype(np.float32), row_idx

